# revision 19
# baseline (speedup 1.0000x reference)
import sys
sys.path.insert(0, "/opt/trn_rl_repo")
import numpy as np

N_ATOMS = 10000
N_SPECIES = 8
N_STRUCT = 8
C = 16
N_BASIS = 8
L_MAX = 3
CUTOFF = 5.0
NCORES = 8
NC_AT = N_ATOMS // NCORES
CNT_MAX = 6
JC = CNT_MAX * C  # 96

_prog_cache = {}
PROFILE = False
LAST_PROF = []

# mn row order within a 64-row block: (l, m, n), n fastest
_LOF = np.repeat(np.arange(4), [(2 * l + 1) * 4 for l in range(4)])
_MOF = np.concatenate([np.repeat(np.arange(2 * l + 1), 4) for l in range(4)])
_NOF = np.concatenate([np.tile(np.arange(4), 2 * l + 1) for l in range(4)])
_SFAC = np.repeat([1.0 / np.sqrt(2.0 * l + 1.0) for l in range(4)],
                  [(2 * l + 1) * 4 for l in range(4)]).astype(np.float64)
# device emits raw sh polynomials; true sh = t[m] * raw (sign irrelevant,
# squares only). t^2 folded into SW/SE stationaries host-side.
_T = np.array([0.28209479,
               0.48860251, 0.48860251, 0.48860251,
               1.09254843, 1.09254843, 3 * 0.31539157, 1.09254843,
               0.54627422,
               3 * 0.59004359, 2.89061144, 5 * 0.45704579,
               5 * 0.37317633, 5 * 0.45704579, 1.44530572, 0.59004359],
              np.float64)
_GM = (_LOF * _LOF + _MOF)  # global m index per mn row
_TSQ = (_T[_GM] ** 2).astype(np.float64)


def _pack(senders, receivers):
    """FFD pack receiver atoms into blocks (<=64 edges, <=CNT_MAX atoms);
    pair blocks; block b of a pair owns edge rows [64b, 64b+edges)."""
    recv = np.asarray(receivers).astype(np.int64)
    send = np.asarray(senders).astype(np.int64)
    order = np.argsort(recv, kind="stable")
    ss = send[order]
    deg = np.bincount(recv, minlength=N_ATOMS)
    starts = np.zeros(N_ATOMS + 1, np.int64)
    starts[1:] = np.cumsum(deg)
    core_blocks = []
    for core in range(NCORES):
        a0 = core * NC_AT
        atoms = sorted(range(a0, a0 + NC_AT), key=lambda a: -deg[a])
        blocks = []
        for a in atoms:
            for blk in blocks:
                if blk[0] + deg[a] <= 64 and len(blk[1]) < CNT_MAX:
                    blk[1].append(a); blk[0] += deg[a]; break
            else:
                blocks.append([deg[a], [a]])
        core_blocks.append(blocks)
    KP = max((len(b) + 1) // 2 for b in core_blocks)
    if KP % 4:
        KP += 4 - KP % 4
    tabs = []
    for core in range(NCORES):
        blocks = core_blocks[core]
        slot_send = np.zeros((128, KP), np.int64)
        slot_val = np.zeros((128, KP), bool)
        MS = np.zeros((128, KP, CNT_MAX), np.float32)
        amap = np.full((2, KP, CNT_MAX), -1, np.int64)
        for bi, (ecnt, atoms) in enumerate(blocks):
            kp, b = bi // 2, bi % 2
            row = 64 * b
            for j, a in enumerate(atoms):
                s0, s1 = starts[a], starts[a + 1]
                n = s1 - s0
                slot_send[row:row + n, kp] = ss[s0:s1]
                slot_val[row:row + n, kp] = True
                MS[row:row + n, kp, j] = 1.0
                amap[b, kp, j] = a
                row += n
            assert row <= 64 * b + 64
        tabs.append(dict(slot_send=slot_send, slot_val=slot_val,
                         MS=MS, amap=amap))
    return KP, tabs


def _emit_scatter(nc, mybir, ppa, PT, G, AS2, k0, k1):
    """Scatter matmuls + transposed squares for kp in [k0, k1),
    ragged groups of 4."""
    AF = mybir.ActivationFunctionType
    f32 = mybir.dt.float32
    g0 = k0
    while g0 < k1:
        n = min(4, k1 - g0)
        pa = ppa.tile([128, 4 * JC], f32, tag="pa")
        for q in range(n):
            kp = g0 + q
            nc.tensor.matmul(pa[:, q * JC:(q + 1) * JC],
                             PT[:, kp, :], G[:, kp, :],
                             start=True, stop=True)
        dst = AS2[:, :, g0 * CNT_MAX:(g0 + n) * CNT_MAX].rearrange(
            "p c (k j) -> p k c j", k=n)
        nc.scalar.activation(
            dst, pa[:, 0:n * JC].rearrange("p (k c j) -> p k c j",
                                           k=n, c=16),
            AF.Square)
        g0 += n


def _build_A(KP):
    import concourse.bass as bass
    import concourse.bacc as bacc
    import concourse.tile as tile
    from concourse import mybir

    f32 = mybir.dt.float32
    f16 = mybir.dt.float16
    ALU = mybir.AluOpType
    AF = mybir.ActivationFunctionType
    KPC = KP * CNT_MAX
    H2 = KP // 2
    Q4 = KP // 4

    nc = bacc.Bacc("TRN2", target_bir_lowering=False, debug=False,
                   num_devices=NCORES)
    PP_d = nc.dram_tensor("pp", [128, KP, 6], f32, kind="ExternalInput").ap()
    WR_d = nc.dram_tensor("wrb", [128, 8, 16], f32,
                          kind="ExternalInput").ap()
    GX_d = nc.dram_tensor("gx", [128, KP, JC], f16,
                          kind="ExternalInput").ap()
    SW_d = nc.dram_tensor("sw", [128, 16, 32], f16, kind="ExternalInput").ap()
    CE_d = nc.dram_tensor("ce", [32, KPC], f32, kind="ExternalInput").ap()
    PTZ_d = nc.dram_tensor("ptz", [128, KP, 64], f16,
                           kind="ExternalInput").ap()
    PTF_d = nc.dram_tensor("ptf", [128, KP, 128], f16,
                           kind="ExternalOutput").ap()
    OUTH_d = nc.dram_tensor("outh", [32, KPC], f32,
                            kind="ExternalOutput").ap()

    with tile.TileContext(nc) as tc:
        with tc.tile_pool(name="main", bufs=1) as pool, \
             tc.tile_pool(name="pa", bufs=3, space="PSUM") as ppa, \
             tc.tile_pool(name="ph", bufs=2, space="PSUM") as pph:
            PP = pool.tile([128, KP, 6], f32, tag="pp")
            WR = pool.tile([128, 8, 16], f32, tag="wr")
            GX = pool.tile([128, KP, JC], f16, tag="g")
            SW = pool.tile([128, 16, 32], f16, tag="sw")
            CE = pool.tile([32, KPC], f32, tag="ce")
            nc.sync.dma_start(PP[:], PP_d[:])
            nc.sync.dma_start(WR[:], WR_d[:])
            for q in range(4):
                sl = slice(q * Q4, (q + 1) * Q4)
                nc.sync.dma_start(GX[:, sl], GX_d[:, sl])
            nc.sync.dma_start(SW[:], SW_d[:])
            nc.sync.dma_start(CE[:], CE_d[:])

            RV = pool.tile([128, KP, 3], f32, tag="rv")
            U = pool.tile([128, KP, 3], f32, tag="u")
            SC = pool.tile([128, KP, 12], f32, tag="sc")
            SH = pool.tile([128, KP, 12], f32, tag="sh")  # raw sh m=4..15
            RR = pool.tile([128, KP, 16], f32, tag="rr")
            TM = pool.tile([128, KP, 16], f32, tag="pp2")
            RG0 = pool.tile([128, KP, 4], f32, tag="rg0")
            PT = pool.tile([128, KP, 128], f16, tag="pt")
            AS2 = pool.tile([128, 16, KPC], f16, tag="as")
            OUTH = pool.tile([32, KPC], f32, tag="oh")

            def sc(i):
                return SC[:, :, i]

            TT = nc.vector.tensor_tensor
            TS = nc.vector.tensor_scalar

            # off-diagonal PT blocks zeroed from DRAM (diag written by TTs)
            for q in range(4):
                sl = slice(q * Q4, (q + 1) * Q4)
                nc.sync.dma_start(PT[0:64, sl, 64:128], PTZ_d[0:64, sl, :])
                nc.sync.dma_start(PT[64:128, sl, 0:64],
                                  PTZ_d[64:128, sl, :])

            # geometry (positions pre-permuted host-side to (y,z,x))
            TT(RV[:], PP[:, :, 3:6], PP[:, :, 0:3], ALU.subtract)
            nc.vector.tensor_mul(U[:], RV[:], RV[:])
            nc.vector.reduce_sum(SC[:, :, 0:1], U[:], mybir.AxisListType.X)
            nc.scalar.activation(sc(1), sc(0), AF.Sqrt)          # r
            nc.vector.tensor_scalar_max(sc(2), sc(1), 1e-6)      # rc
            nc.vector.reciprocal(sc(3), sc(2))                   # rinv
            TT(U[:], RV[:], SC[:, :, 3:4].to_broadcast([128, KP, 3]),
               ALU.mult)
            # fc = 0.5*cos(pi*min(r,5)/5)+0.5 ;  cos(x) = -sin(x - pi/2)
            nc.vector.tensor_scalar_min(sc(6), sc(1), CUTOFF)
            TS(sc(6), sc(6), float(np.pi / CUTOFF), float(-np.pi / 2),
               ALU.mult, ALU.add)
            nc.scalar.activation(sc(7), sc(6), AF.Sin)
            TS(sc(4), sc(7), -0.5, 0.5, ALU.mult, ALU.add)       # fc
            TT(sc(5), sc(4), sc(3), ALU.mult)
            nc.vector.tensor_scalar_mul(sc(5), sc(5),
                                        float(np.sqrt(2.0 / CUTOFF)))  # g
            y, z, x = U[:, :, 0], U[:, :, 1], U[:, :, 2]
            x2, y2, z2, xy, yz, xz = (sc(i) for i in (6, 7, 8, 9, 10, 11))
            nc.vector.tensor_mul(x2, x, x)
            nc.vector.tensor_mul(y2, y, y)
            nc.vector.tensor_mul(z2, z, z)
            nc.vector.tensor_mul(xy, x, y)
            nc.vector.tensor_mul(yz, y, z)
            nc.vector.tensor_mul(xz, x, z)

            # raw sh m=4..15 -> SH cols 0..11
            def shm(m):
                return SH[:, :, m - 4]

            nc.scalar.copy(shm(4), xy)
            nc.scalar.copy(shm(5), yz)
            nc.vector.tensor_scalar_add(shm(6), z2, -1.0 / 3.0)
            nc.scalar.copy(shm(7), xz)
            d_, t_ = sc(0), sc(1)
            TT(d_, x2, y2, ALU.subtract)                  # x2-y2
            nc.scalar.copy(shm(8), d_)
            nc.vector.scalar_tensor_tensor(t_, y2, 1.0 / 3.0, x2,
                                           ALU.mult, ALU.subtract)
            TT(shm(9), t_, y, ALU.mult)                   # y*(y2/3-x2)
            TT(shm(10), xy, z, ALU.mult)                  # xyz
            nc.vector.tensor_scalar_add(t_, z2, -0.2)
            TT(shm(11), t_, y, ALU.mult)                  # y*(z2-1/5)
            TT(shm(13), t_, x, ALU.mult)                  # x*(z2-1/5)
            nc.vector.tensor_scalar_add(t_, z2, -0.6)
            TT(shm(12), t_, z, ALU.mult)                  # z*(z2-3/5)
            TT(shm(14), d_, z, ALU.mult)                  # z*(x2-y2)
            nc.vector.scalar_tensor_tensor(t_, y2, 3.0, x2,
                                           ALU.mult, ALU.subtract)
            TT(shm(15), t_, x, ALU.mult)                  # x*(3y2-x2)
            # fold g into SH and U (radial then accumulates raw sin terms)
            gb12 = SC[:, :, 5:6].to_broadcast([128, KP, 12])
            TT(SH[:], SH[:], gb12, ALU.mult)
            TT(U[:], U[:], SC[:, :, 5:6].to_broadcast([128, KP, 3]),
               ALU.mult)

            # radial: s_b recurrence with inline accumulate (raw, no g)
            C2, SA, SB, TP = sc(0), sc(1), sc(3), sc(4)
            TS(sc(6), sc(2), float(np.pi / CUTOFF), float(-np.pi),
               ALU.mult, ALU.add)
            nc.scalar.activation(sc(7), sc(6), AF.Sin)
            nc.vector.tensor_scalar_mul(SA, sc(7), -1.0)          # s1
            TS(sc(6), sc(2), float(np.pi / CUTOFF), float(-np.pi / 2),
               ALU.mult, ALU.add)
            nc.scalar.activation(sc(7), sc(6), AF.Sin)
            nc.vector.tensor_scalar_mul(C2, sc(7), -2.0)          # 2cos
            for b in range(1, N_BASIS + 1):
                if b == 1:
                    cur = SA
                elif b == 2:
                    TT(SB, C2, SA, ALU.mult)
                    cur = SB
                else:
                    TT(TP, C2, SB if b % 2 else SA, ALU.mult)
                    dst = SA if b % 2 else SB
                    TT(dst, TP, SA if b % 2 else SB, ALU.subtract)
                    cur = dst
                bb = cur.unsqueeze(2).to_broadcast([128, KP, 16])
                wb = WR[:, b - 1:b, :].to_broadcast([128, KP, 16])
                if b == 1:
                    TT(RR[:], bb, wb, ALU.mult)
                else:
                    TT(TM[:], bb, wb, ALU.mult)
                    TT(RR[:], RR[:], TM[:], ALU.add)
            # l0 needs g*RR (g folded into SH/U covers l1..l3 only)
            TT(RG0[:], RR[:, :, 0:4],
               SC[:, :, 5:6].to_broadcast([128, KP, 4]), ALU.mult)

            def quarter_chain(q):
                sl = slice(q * Q4, (q + 1) * Q4)
                # write PT diagonal blocks directly (partition-split)
                for hp in range(2):
                    pr = slice(hp * 64, (hp + 1) * 64)
                    co = hp * 64
                    nc.scalar.copy(PT[pr, sl, co:co + 4], RG0[pr, sl, :])
                    TT(PT[pr, sl, co + 4:co + 16].rearrange(
                        "p k (m n) -> p k m n", n=4),
                       U[pr, sl, :].unsqueeze(3).to_broadcast(
                           [64, Q4, 3, 4]),
                       RR[pr, sl, 4:8].unsqueeze(2).to_broadcast(
                           [64, Q4, 3, 4]),
                       ALU.mult)
                    TT(PT[pr, sl, co + 16:co + 36].rearrange(
                        "p k (m n) -> p k m n", n=4),
                       SH[pr, sl, 0:5].unsqueeze(3).to_broadcast(
                           [64, Q4, 5, 4]),
                       RR[pr, sl, 8:12].unsqueeze(2).to_broadcast(
                           [64, Q4, 5, 4]),
                       ALU.mult)
                    TT(PT[pr, sl, co + 36:co + 64].rearrange(
                        "p k (m n) -> p k m n", n=4),
                       SH[pr, sl, 5:12].unsqueeze(3).to_broadcast(
                           [64, Q4, 7, 4]),
                       RR[pr, sl, 12:16].unsqueeze(2).to_broadcast(
                           [64, Q4, 7, 4]),
                       ALU.mult)
                nc.sync.dma_start(PTF_d[:, sl, :], PT[:, sl, :])

            phs = []
            for q in range(4):
                quarter_chain(q)
                _emit_scatter(nc, mybir, ppa, PT, GX, AS2,
                              q * Q4, (q + 1) * Q4)
                if q % 2 == 1:
                    h = q // 2
                    ph = pph.tile([32, H2 * CNT_MAX], f32, tag="ph")
                    for c in range(16):
                        nc.tensor.matmul(
                            ph[:], SW[:, c, :],
                            AS2[:, c,
                                h * H2 * CNT_MAX:(h + 1) * H2 * CNT_MAX],
                            start=(c == 0), stop=(c == 15))
                    phs.append(ph)
            # h1 = h1_pre * cemb (vector; emitted last, no pipeline stall)
            for h, ph in enumerate(phs):
                cs = h * H2 * CNT_MAX
                TT(OUTH[:, cs:cs + H2 * CNT_MAX], ph[:],
                   CE[:, cs:cs + H2 * CNT_MAX], ALU.mult)
                nc.sync.dma_start(OUTH_d[:, cs:cs + H2 * CNT_MAX],
                                  OUTH[:, cs:cs + H2 * CNT_MAX])
    nc.compile()
    return nc


def _build_B(KP):
    import concourse.bass as bass
    import concourse.bacc as bacc
    import concourse.tile as tile
    from concourse import mybir

    f32 = mybir.dt.float32
    f16 = mybir.dt.float16
    ALU = mybir.AluOpType
    KPC = KP * CNT_MAX
    H2 = KP // 2
    Q4 = KP // 4

    nc = bacc.Bacc("TRN2", target_bir_lowering=False, debug=False,
                   num_devices=NCORES)
    PTF_d = nc.dram_tensor("ptf", [128, KP, 128], f16,
                           kind="ExternalInput").ap()
    GX_d = nc.dram_tensor("gx", [128, KP, JC], f16,
                          kind="ExternalInput").ap()
    SE_d = nc.dram_tensor("se", [128, 16, 2], f16, kind="ExternalInput").ap()
    OUTE_d = nc.dram_tensor("oute", [2, KPC], f32,
                            kind="ExternalOutput").ap()

    with tile.TileContext(nc) as tc:
        with tc.tile_pool(name="main", bufs=1) as pool, \
             tc.tile_pool(name="pa", bufs=3, space="PSUM") as ppa, \
             tc.tile_pool(name="ph", bufs=2, space="PSUM") as pph:
            GX = pool.tile([128, KP, JC], f16, tag="g")
            SE = pool.tile([128, 16, 2], f16, tag="se")
            PT = pool.tile([128, KP, 128], f16, tag="pt")
            AS2 = pool.tile([128, 16, KPC], f16, tag="as")
            OUTE = pool.tile([2, KPC], f32, tag="oe")

            nc.sync.dma_start(SE[:], SE_d[:])
            for q in range(4):
                sl = slice(q * Q4, (q + 1) * Q4)
                nc.sync.dma_start(GX[:, sl], GX_d[:, sl])
                nc.sync.dma_start(PT[:, sl, :], PTF_d[:, sl, :])
                _emit_scatter(nc, mybir, ppa, PT, GX, AS2,
                              q * Q4, (q + 1) * Q4)
                if q % 2 == 1:
                    h = q // 2
                    pe = pph.tile([2, H2 * CNT_MAX], f32, tag="pe")
                    for c in range(16):
                        nc.tensor.matmul(
                            pe[:], SE[:, c, :],
                            AS2[:, c,
                                h * H2 * CNT_MAX:(h + 1) * H2 * CNT_MAX],
                            start=(c == 0), stop=(c == 15))
                    cs = h * H2 * CNT_MAX
                    nc.scalar.copy(OUTE[:, cs:cs + H2 * CNT_MAX], pe[:])
                    nc.sync.dma_start(OUTE_d[:, cs:cs + H2 * CNT_MAX],
                                      OUTE[:, cs:cs + H2 * CNT_MAX])
    nc.compile()
    return nc


def kernel(positions, embed, W_rad, W_inv1, W_inv2, w_out, comp_weights,
           senders, receivers, species, structure_ids):
    from concourse import bass_utils

    positions = np.asarray(positions, np.float32)
    embed = np.asarray(embed, np.float32)
    W_rad = np.asarray(W_rad, np.float32)
    W_inv1 = np.asarray(W_inv1, np.float32)
    W_inv2 = np.asarray(W_inv2, np.float32)
    w_out = np.asarray(w_out, np.float32)
    comp_weights = np.asarray(comp_weights, np.float32)
    senders = np.asarray(senders).astype(np.int64)
    receivers = np.asarray(receivers).astype(np.int64)
    species = np.asarray(species).astype(np.int64)
    structure_ids_np = np.asarray(structure_ids).astype(np.int64)

    KP, tabs = _pack(senders, receivers)
    KPC = KP * CNT_MAX
    if KP not in _prog_cache:
        _prog_cache[KP] = (_build_A(KP), _build_B(KP))
    ncA, ncB = _prog_cache[KP]

    cemb = embed[species]  # [N,16]

    def sw_pack(W):  # [256,16] -> [128, 16, 32] f16
        SW = np.zeros((128, 16, 32), np.float64)
        for b in range(2):
            rows = slice(b * 64, (b + 1) * 64)
            cols = slice(b * 16, (b + 1) * 16)
            for c in range(16):
                SW[rows, c, cols] = ((_SFAC * _TSQ)[:, None] *
                                     W[_LOF * 64 + _NOF * 16 + c, :])
        return SW.astype(np.float16)

    def se_pack(wo):  # [256] -> [128, 16, 2] f16
        SE = np.zeros((128, 16, 2), np.float64)
        for b in range(2):
            for c in range(16):
                SE[b * 64:(b + 1) * 64, c, b] = (
                    _SFAC * _TSQ * wo[_LOF * 64 + _NOF * 16 + c])
        return SE.astype(np.float16)

    SW1 = sw_pack(W_inv1)
    SE2 = se_pack(w_out)
    WRB = np.zeros((8, 16), np.float32)
    for l in range(L_MAX + 1):
        WRB[:, l * 4:(l + 1) * 4] = W_rad[l]
    WRB = np.broadcast_to(WRB[None], (128, 8, 16)).copy()

    def gx_pack(hsrc, tb):
        """GX[p,kp,c*6+j] = h[send[p,kp],c] * MS[p,kp,j], f16."""
        sl, val, MS = tb["slot_send"], tb["slot_val"], tb["MS"]
        hs = hsrc[np.where(val, sl, 0)]
        hs[~val] = 0.0
        gx = hs[:, :, :, None] * MS[:, :, None, :]
        return gx.reshape(128, -1, JC).astype(np.float16)

    PERM = np.array([1, 2, 0])  # (x,y,z) -> (y,z,x)
    PTZ = np.zeros((128, KP, 64), np.float16)
    maps1 = []
    for core in range(NCORES):
        tb = tabs[core]
        sl, val = tb["slot_send"], tb["slot_val"]
        amap = tb["amap"]
        jidx = tb["MS"].argmax(2)
        bidx = (np.arange(128)[:, None] // 64) * np.ones(
            (1, KP), np.int64)
        ratom = amap[bidx, np.arange(KP)[None, :], jidx]
        ratom = np.where(val, ratom, 0)
        satom = np.where(val, sl, 0)
        pp = np.zeros((128, KP, 6), np.float32)
        pp[:, :, 0:3] = positions[satom][:, :, PERM]
        pp[:, :, 3:6] = positions[ratom][:, :, PERM]
        ce = np.zeros((32, KPC), np.float32)
        av = amap.reshape(2, KPC)
        for b in range(2):
            valid = av[b] >= 0
            ce[b * 16:(b + 1) * 16, valid] = cemb[av[b][valid]].T
        maps1.append(dict(pp=pp, wrb=WRB, gx=gx_pack(cemb, tb),
                          sw=SW1, ce=ce, ptz=PTZ))

    resA = bass_utils.run_bass_kernel_spmd(ncA, maps1,
                                           core_ids=list(range(NCORES)),
                                           trace=PROFILE)
    if PROFILE:
        LAST_PROF.append(resA)

    h1_full = np.zeros((N_ATOMS, C), np.float32)
    for core in range(NCORES):
        amap = tabs[core]["amap"].reshape(2, KPC)
        outh = resA.results[core]["outh"]  # [32, KPC]
        for b in range(2):
            valid = amap[b] >= 0
            h1_full[amap[b][valid]] = outh[b * 16:(b + 1) * 16, valid].T

    maps2 = []
    for core in range(NCORES):
        tb = tabs[core]
        maps2.append(dict(ptf=resA.results[core]["ptf"],
                          gx=gx_pack(h1_full, tb), se=SE2))
    resB = bass_utils.run_bass_kernel_spmd(ncB, maps2,
                                           core_ids=list(range(NCORES)),
                                           trace=PROFILE)
    if PROFILE:
        LAST_PROF.append(resB)

    e_atom = np.zeros(N_ATOMS, np.float32)
    for core in range(NCORES):
        amap = tabs[core]["amap"].reshape(2, KPC)
        oute = resB.results[core]["oute"]  # [2, KPC]
        for b in range(2):
            valid = amap[b] >= 0
            e_atom[amap[b][valid]] = oute[b, valid]
    e_atom += comp_weights[species]
    out = np.zeros(N_STRUCT, np.float32)
    np.add.at(out, structure_ids_np, e_atom)
    return out


# revision 20
# speedup vs baseline: 1.0151x; 1.0151x over previous
import sys
sys.path.insert(0, "/opt/trn_rl_repo")
import numpy as np

N_ATOMS = 10000
N_SPECIES = 8
N_STRUCT = 8
C = 16
N_BASIS = 8
L_MAX = 3
CUTOFF = 5.0
NCORES = 8
NC_AT = N_ATOMS // NCORES
CNT_MAX = 6
JC = CNT_MAX * C  # 96

_prog_cache = {}
PROFILE = False
LAST_PROF = []

# mn row order within a 64-row block: (l, m, n), n fastest
_LOF = np.repeat(np.arange(4), [(2 * l + 1) * 4 for l in range(4)])
_MOF = np.concatenate([np.repeat(np.arange(2 * l + 1), 4) for l in range(4)])
_NOF = np.concatenate([np.tile(np.arange(4), 2 * l + 1) for l in range(4)])
_SFAC = np.repeat([1.0 / np.sqrt(2.0 * l + 1.0) for l in range(4)],
                  [(2 * l + 1) * 4 for l in range(4)]).astype(np.float64)
# device emits raw sh polynomials; true sh = t[m] * raw (sign irrelevant,
# squares only). t^2 folded into SW/SE stationaries host-side.
_T = np.array([0.28209479,
               0.48860251, 0.48860251, 0.48860251,
               1.09254843, 1.09254843, 3 * 0.31539157, 1.09254843,
               0.54627422,
               3 * 0.59004359, 2.89061144, 5 * 0.45704579,
               5 * 0.37317633, 5 * 0.45704579, 1.44530572, 0.59004359],
              np.float64)
_GM = (_LOF * _LOF + _MOF)  # global m index per mn row
_TSQ = (_T[_GM] ** 2).astype(np.float64)


def _pack(senders, receivers):
    """FFD pack receiver atoms into blocks (<=64 edges, <=CNT_MAX atoms);
    pair blocks; block b of a pair owns edge rows [64b, 64b+edges)."""
    recv = np.asarray(receivers).astype(np.int64)
    send = np.asarray(senders).astype(np.int64)
    order = np.argsort(recv, kind="stable")
    ss = send[order]
    deg = np.bincount(recv, minlength=N_ATOMS)
    starts = np.zeros(N_ATOMS + 1, np.int64)
    starts[1:] = np.cumsum(deg)
    core_blocks = []
    for core in range(NCORES):
        a0 = core * NC_AT
        atoms = sorted(range(a0, a0 + NC_AT), key=lambda a: -deg[a])
        blocks = []
        for a in atoms:
            for blk in blocks:
                if blk[0] + deg[a] <= 64 and len(blk[1]) < CNT_MAX:
                    blk[1].append(a); blk[0] += deg[a]; break
            else:
                blocks.append([deg[a], [a]])
        core_blocks.append(blocks)
    KP = max((len(b) + 1) // 2 for b in core_blocks)
    if KP % 4:
        KP += 4 - KP % 4
    tabs = []
    for core in range(NCORES):
        blocks = core_blocks[core]
        slot_send = np.zeros((128, KP), np.int64)
        slot_val = np.zeros((128, KP), bool)
        MS = np.zeros((128, KP, CNT_MAX), np.float32)
        amap = np.full((2, KP, CNT_MAX), -1, np.int64)
        for bi, (ecnt, atoms) in enumerate(blocks):
            kp, b = bi // 2, bi % 2
            row = 64 * b
            for j, a in enumerate(atoms):
                s0, s1 = starts[a], starts[a + 1]
                n = s1 - s0
                slot_send[row:row + n, kp] = ss[s0:s1]
                slot_val[row:row + n, kp] = True
                MS[row:row + n, kp, j] = 1.0
                amap[b, kp, j] = a
                row += n
            assert row <= 64 * b + 64
        tabs.append(dict(slot_send=slot_send, slot_val=slot_val,
                         MS=MS, amap=amap))
    return KP, tabs


def _emit_scatter(nc, mybir, ppa, PT, G, AS2, k0, k1):
    """Scatter matmuls + transposed squares for kp in [k0, k1),
    ragged groups of 4."""
    AF = mybir.ActivationFunctionType
    f32 = mybir.dt.float32
    g0 = k0
    while g0 < k1:
        n = min(4, k1 - g0)
        pa = ppa.tile([128, 4 * JC], f32, tag="pa")
        for q in range(n):
            kp = g0 + q
            nc.tensor.matmul(pa[:, q * JC:(q + 1) * JC],
                             PT[:, kp, :], G[:, kp, :],
                             start=True, stop=True)
        dst = AS2[:, :, g0 * CNT_MAX:(g0 + n) * CNT_MAX].rearrange(
            "p c (k j) -> p k c j", k=n)
        nc.scalar.activation(
            dst, pa[:, 0:n * JC].rearrange("p (k c j) -> p k c j",
                                           k=n, c=16),
            AF.Square)
        g0 += n


def _build_A(KP):
    import concourse.bass as bass
    import concourse.bacc as bacc
    import concourse.tile as tile
    from concourse import mybir

    f32 = mybir.dt.float32
    f16 = mybir.dt.float16
    ALU = mybir.AluOpType
    AF = mybir.ActivationFunctionType
    KPC = KP * CNT_MAX
    H2 = KP // 2
    Q4 = KP // 4

    nc = bacc.Bacc("TRN2", target_bir_lowering=False, debug=False,
                   num_devices=NCORES)
    PP_d = nc.dram_tensor("pp", [128, KP, 6], f32, kind="ExternalInput").ap()
    WR_d = nc.dram_tensor("wrb", [128, 8, 16], f32,
                          kind="ExternalInput").ap()
    GX_d = nc.dram_tensor("gx", [128, KP, JC], f16,
                          kind="ExternalInput").ap()
    SW_d = nc.dram_tensor("sw", [128, 16, 32], f16, kind="ExternalInput").ap()
    CE_d = nc.dram_tensor("ce", [32, KPC], f32, kind="ExternalInput").ap()
    PTZ_d = nc.dram_tensor("ptz", [128, KP, 128], f16,
                           kind="ExternalInput").ap()
    PTF_d = nc.dram_tensor("ptf", [128, KP, 128], f16,
                           kind="ExternalOutput").ap()
    OUTH_d = nc.dram_tensor("outh", [32, KPC], f32,
                            kind="ExternalOutput").ap()

    with tile.TileContext(nc) as tc:
        with tc.tile_pool(name="main", bufs=1) as pool, \
             tc.tile_pool(name="pa", bufs=3, space="PSUM") as ppa, \
             tc.tile_pool(name="ph", bufs=2, space="PSUM") as pph:
            PP = pool.tile([128, KP, 6], f32, tag="pp")
            WR = pool.tile([128, 8, 16], f32, tag="wr")
            GX = pool.tile([128, KP, JC], f16, tag="g")
            SW = pool.tile([128, 16, 32], f16, tag="sw")
            CE = pool.tile([32, KPC], f32, tag="ce")
            nc.sync.dma_start(PP[:], PP_d[:])
            nc.sync.dma_start(WR[:], WR_d[:])
            for q in range(4):
                sl = slice(q * Q4, (q + 1) * Q4)
                nc.sync.dma_start(GX[:, sl], GX_d[:, sl])
            nc.sync.dma_start(SW[:], SW_d[:])
            nc.sync.dma_start(CE[:], CE_d[:])

            RV = pool.tile([128, KP, 3], f32, tag="rv")
            U = pool.tile([128, KP, 3], f32, tag="u")
            SC = pool.tile([128, KP, 12], f32, tag="sc")
            SH = pool.tile([128, KP, 12], f32, tag="sh")  # raw sh m=4..15
            RR = pool.tile([128, KP, 16], f32, tag="rr")
            TM = pool.tile([128, KP, 16], f32, tag="pp2")
            RG0 = pool.tile([128, KP, 4], f32, tag="rg0")
            PT = pool.tile([128, KP, 128], f16, tag="pt")
            AS2 = pool.tile([128, 16, KPC], f16, tag="as")
            OUTH = pool.tile([32, KPC], f32, tag="oh")

            def sc(i):
                return SC[:, :, i]

            TT = nc.vector.tensor_tensor
            TS = nc.vector.tensor_scalar

            # PT slab arrives pre-zeroed from DRAM (diag overwritten by TTs)
            for q in range(4):
                sl = slice(q * Q4, (q + 1) * Q4)
                nc.sync.dma_start(PT[:, sl, :], PTZ_d[:, sl, :])

            # geometry (positions pre-permuted host-side to (y,z,x))
            TT(RV[:], PP[:, :, 3:6], PP[:, :, 0:3], ALU.subtract)
            nc.vector.tensor_mul(U[:], RV[:], RV[:])
            nc.vector.reduce_sum(SC[:, :, 0:1], U[:], mybir.AxisListType.X)
            nc.scalar.activation(sc(1), sc(0), AF.Sqrt)          # r
            nc.vector.tensor_scalar_max(sc(2), sc(1), 1e-6)      # rc
            nc.vector.reciprocal(sc(3), sc(2))                   # rinv
            TT(U[:], RV[:], SC[:, :, 3:4].to_broadcast([128, KP, 3]),
               ALU.mult)
            # fc = 0.5*cos(pi*min(r,5)/5)+0.5 ;  cos(x) = -sin(x - pi/2)
            nc.vector.tensor_scalar_min(sc(6), sc(1), CUTOFF)
            TS(sc(6), sc(6), float(np.pi / CUTOFF), float(-np.pi / 2),
               ALU.mult, ALU.add)
            nc.scalar.activation(sc(7), sc(6), AF.Sin)
            TS(sc(4), sc(7), -0.5, 0.5, ALU.mult, ALU.add)       # fc
            TT(sc(5), sc(4), sc(3), ALU.mult)
            nc.vector.tensor_scalar_mul(sc(5), sc(5),
                                        float(np.sqrt(2.0 / CUTOFF)))  # g
            y, z, x = U[:, :, 0], U[:, :, 1], U[:, :, 2]
            x2, y2, z2, xy, yz, xz = (sc(i) for i in (6, 7, 8, 9, 10, 11))
            nc.vector.tensor_mul(x2, x, x)
            nc.vector.tensor_mul(y2, y, y)
            nc.vector.tensor_mul(z2, z, z)
            nc.vector.tensor_mul(xy, x, y)
            nc.vector.tensor_mul(yz, y, z)
            nc.vector.tensor_mul(xz, x, z)

            # raw sh m=4..15 -> SH cols 0..11
            def shm(m):
                return SH[:, :, m - 4]

            nc.scalar.copy(shm(4), xy)
            nc.scalar.copy(shm(5), yz)
            nc.vector.tensor_scalar_add(shm(6), z2, -1.0 / 3.0)
            nc.scalar.copy(shm(7), xz)
            d_, t_ = sc(0), sc(1)
            TT(d_, x2, y2, ALU.subtract)                  # x2-y2
            nc.scalar.copy(shm(8), d_)
            nc.vector.scalar_tensor_tensor(t_, y2, 1.0 / 3.0, x2,
                                           ALU.mult, ALU.subtract)
            TT(shm(9), t_, y, ALU.mult)                   # y*(y2/3-x2)
            TT(shm(10), xy, z, ALU.mult)                  # xyz
            nc.vector.tensor_scalar_add(t_, z2, -0.2)
            TT(shm(11), t_, y, ALU.mult)                  # y*(z2-1/5)
            TT(shm(13), t_, x, ALU.mult)                  # x*(z2-1/5)
            nc.vector.tensor_scalar_add(t_, z2, -0.6)
            TT(shm(12), t_, z, ALU.mult)                  # z*(z2-3/5)
            TT(shm(14), d_, z, ALU.mult)                  # z*(x2-y2)
            nc.vector.scalar_tensor_tensor(t_, y2, 3.0, x2,
                                           ALU.mult, ALU.subtract)
            TT(shm(15), t_, x, ALU.mult)                  # x*(3y2-x2)
            # fold g into SH and U (radial then accumulates raw sin terms)
            gb12 = SC[:, :, 5:6].to_broadcast([128, KP, 12])
            TT(SH[:], SH[:], gb12, ALU.mult)
            TT(U[:], U[:], SC[:, :, 5:6].to_broadcast([128, KP, 3]),
               ALU.mult)

            # radial: s_b recurrence with inline accumulate (raw, no g)
            C2, SA, SB, TP = sc(0), sc(1), sc(3), sc(4)
            TS(sc(6), sc(2), float(np.pi / CUTOFF), float(-np.pi),
               ALU.mult, ALU.add)
            nc.scalar.activation(sc(7), sc(6), AF.Sin)
            nc.vector.tensor_scalar_mul(SA, sc(7), -1.0)          # s1
            TS(sc(6), sc(2), float(np.pi / CUTOFF), float(-np.pi / 2),
               ALU.mult, ALU.add)
            nc.scalar.activation(sc(7), sc(6), AF.Sin)
            nc.vector.tensor_scalar_mul(C2, sc(7), -2.0)          # 2cos
            for b in range(1, N_BASIS + 1):
                if b == 1:
                    cur = SA
                elif b == 2:
                    TT(SB, C2, SA, ALU.mult)
                    cur = SB
                else:
                    TT(TP, C2, SB if b % 2 else SA, ALU.mult)
                    dst = SA if b % 2 else SB
                    TT(dst, TP, SA if b % 2 else SB, ALU.subtract)
                    cur = dst
                bb = cur.unsqueeze(2).to_broadcast([128, KP, 16])
                wb = WR[:, b - 1:b, :].to_broadcast([128, KP, 16])
                if b == 1:
                    TT(RR[:], bb, wb, ALU.mult)
                else:
                    TT(TM[:], bb, wb, ALU.mult)
                    TT(RR[:], RR[:], TM[:], ALU.add)
            # l0 needs g*RR (g folded into SH/U covers l1..l3 only)
            TT(RG0[:], RR[:, :, 0:4],
               SC[:, :, 5:6].to_broadcast([128, KP, 4]), ALU.mult)

            def quarter_chain(q):
                sl = slice(q * Q4, (q + 1) * Q4)
                # write PT diagonal blocks directly (partition-split)
                for hp in range(2):
                    pr = slice(hp * 64, (hp + 1) * 64)
                    co = hp * 64
                    nc.scalar.copy(PT[pr, sl, co:co + 4], RG0[pr, sl, :])
                    TT(PT[pr, sl, co + 4:co + 16].rearrange(
                        "p k (m n) -> p k m n", n=4),
                       U[pr, sl, :].unsqueeze(3).to_broadcast(
                           [64, Q4, 3, 4]),
                       RR[pr, sl, 4:8].unsqueeze(2).to_broadcast(
                           [64, Q4, 3, 4]),
                       ALU.mult)
                    TT(PT[pr, sl, co + 16:co + 36].rearrange(
                        "p k (m n) -> p k m n", n=4),
                       SH[pr, sl, 0:5].unsqueeze(3).to_broadcast(
                           [64, Q4, 5, 4]),
                       RR[pr, sl, 8:12].unsqueeze(2).to_broadcast(
                           [64, Q4, 5, 4]),
                       ALU.mult)
                    TT(PT[pr, sl, co + 36:co + 64].rearrange(
                        "p k (m n) -> p k m n", n=4),
                       SH[pr, sl, 5:12].unsqueeze(3).to_broadcast(
                           [64, Q4, 7, 4]),
                       RR[pr, sl, 12:16].unsqueeze(2).to_broadcast(
                           [64, Q4, 7, 4]),
                       ALU.mult)
                nc.sync.dma_start(PTF_d[:, sl, :], PT[:, sl, :])

            phs = []
            for q in range(4):
                quarter_chain(q)
                _emit_scatter(nc, mybir, ppa, PT, GX, AS2,
                              q * Q4, (q + 1) * Q4)
                if q % 2 == 1:
                    h = q // 2
                    ph = pph.tile([32, H2 * CNT_MAX], f32, tag="ph")
                    for c in range(16):
                        nc.tensor.matmul(
                            ph[:], SW[:, c, :],
                            AS2[:, c,
                                h * H2 * CNT_MAX:(h + 1) * H2 * CNT_MAX],
                            start=(c == 0), stop=(c == 15))
                    phs.append(ph)
            # h1 = h1_pre * cemb (vector; emitted last, no pipeline stall)
            for h, ph in enumerate(phs):
                cs = h * H2 * CNT_MAX
                TT(OUTH[:, cs:cs + H2 * CNT_MAX], ph[:],
                   CE[:, cs:cs + H2 * CNT_MAX], ALU.mult)
                nc.sync.dma_start(OUTH_d[:, cs:cs + H2 * CNT_MAX],
                                  OUTH[:, cs:cs + H2 * CNT_MAX])
    nc.compile()
    return nc


def _build_B(KP):
    import concourse.bass as bass
    import concourse.bacc as bacc
    import concourse.tile as tile
    from concourse import mybir

    f32 = mybir.dt.float32
    f16 = mybir.dt.float16
    ALU = mybir.AluOpType
    KPC = KP * CNT_MAX
    H2 = KP // 2
    Q4 = KP // 4

    nc = bacc.Bacc("TRN2", target_bir_lowering=False, debug=False,
                   num_devices=NCORES)
    PTF_d = nc.dram_tensor("ptf", [128, KP, 128], f16,
                           kind="ExternalInput").ap()
    GX_d = nc.dram_tensor("gx", [128, KP, JC], f16,
                          kind="ExternalInput").ap()
    SE_d = nc.dram_tensor("se", [128, 16, 2], f16, kind="ExternalInput").ap()
    OUTE_d = nc.dram_tensor("oute", [2, KPC], f32,
                            kind="ExternalOutput").ap()

    with tile.TileContext(nc) as tc:
        with tc.tile_pool(name="main", bufs=1) as pool, \
             tc.tile_pool(name="pa", bufs=3, space="PSUM") as ppa, \
             tc.tile_pool(name="ph", bufs=2, space="PSUM") as pph:
            GX = pool.tile([128, KP, JC], f16, tag="g")
            SE = pool.tile([128, 16, 2], f16, tag="se")
            PT = pool.tile([128, KP, 128], f16, tag="pt")
            AS2 = pool.tile([128, 16, KPC], f16, tag="as")
            OUTE = pool.tile([2, KPC], f32, tag="oe")

            nc.sync.dma_start(SE[:], SE_d[:])
            for q in range(4):
                sl = slice(q * Q4, (q + 1) * Q4)
                nc.sync.dma_start(GX[:, sl], GX_d[:, sl])
                nc.sync.dma_start(PT[:, sl, :], PTF_d[:, sl, :])
                _emit_scatter(nc, mybir, ppa, PT, GX, AS2,
                              q * Q4, (q + 1) * Q4)
                if q % 2 == 1:
                    h = q // 2
                    pe = pph.tile([2, H2 * CNT_MAX], f32, tag="pe")
                    for c in range(16):
                        nc.tensor.matmul(
                            pe[:], SE[:, c, :],
                            AS2[:, c,
                                h * H2 * CNT_MAX:(h + 1) * H2 * CNT_MAX],
                            start=(c == 0), stop=(c == 15))
                    cs = h * H2 * CNT_MAX
                    nc.scalar.copy(OUTE[:, cs:cs + H2 * CNT_MAX], pe[:])
                    nc.sync.dma_start(OUTE_d[:, cs:cs + H2 * CNT_MAX],
                                      OUTE[:, cs:cs + H2 * CNT_MAX])
    nc.compile()
    return nc


def kernel(positions, embed, W_rad, W_inv1, W_inv2, w_out, comp_weights,
           senders, receivers, species, structure_ids):
    from concourse import bass_utils

    positions = np.asarray(positions, np.float32)
    embed = np.asarray(embed, np.float32)
    W_rad = np.asarray(W_rad, np.float32)
    W_inv1 = np.asarray(W_inv1, np.float32)
    W_inv2 = np.asarray(W_inv2, np.float32)
    w_out = np.asarray(w_out, np.float32)
    comp_weights = np.asarray(comp_weights, np.float32)
    senders = np.asarray(senders).astype(np.int64)
    receivers = np.asarray(receivers).astype(np.int64)
    species = np.asarray(species).astype(np.int64)
    structure_ids_np = np.asarray(structure_ids).astype(np.int64)

    KP, tabs = _pack(senders, receivers)
    KPC = KP * CNT_MAX
    if KP not in _prog_cache:
        _prog_cache[KP] = (_build_A(KP), _build_B(KP))
    ncA, ncB = _prog_cache[KP]

    cemb = embed[species]  # [N,16]

    def sw_pack(W):  # [256,16] -> [128, 16, 32] f16
        SW = np.zeros((128, 16, 32), np.float64)
        for b in range(2):
            rows = slice(b * 64, (b + 1) * 64)
            cols = slice(b * 16, (b + 1) * 16)
            for c in range(16):
                SW[rows, c, cols] = ((_SFAC * _TSQ)[:, None] *
                                     W[_LOF * 64 + _NOF * 16 + c, :])
        return SW.astype(np.float16)

    def se_pack(wo):  # [256] -> [128, 16, 2] f16
        SE = np.zeros((128, 16, 2), np.float64)
        for b in range(2):
            for c in range(16):
                SE[b * 64:(b + 1) * 64, c, b] = (
                    _SFAC * _TSQ * wo[_LOF * 64 + _NOF * 16 + c])
        return SE.astype(np.float16)

    SW1 = sw_pack(W_inv1)
    SE2 = se_pack(w_out)
    WRB = np.zeros((8, 16), np.float32)
    for l in range(L_MAX + 1):
        WRB[:, l * 4:(l + 1) * 4] = W_rad[l]
    WRB = np.broadcast_to(WRB[None], (128, 8, 16)).copy()

    def gx_pack(hsrc, tb):
        """GX[p,kp,c*6+j] = h[send[p,kp],c] * MS[p,kp,j], f16."""
        sl, val, MS = tb["slot_send"], tb["slot_val"], tb["MS"]
        hs = hsrc[np.where(val, sl, 0)]
        hs[~val] = 0.0
        gx = hs[:, :, :, None] * MS[:, :, None, :]
        return gx.reshape(128, -1, JC).astype(np.float16)

    PERM = np.array([1, 2, 0])  # (x,y,z) -> (y,z,x)
    PTZ = np.zeros((128, KP, 128), np.float16)
    maps1 = []
    for core in range(NCORES):
        tb = tabs[core]
        sl, val = tb["slot_send"], tb["slot_val"]
        amap = tb["amap"]
        jidx = tb["MS"].argmax(2)
        bidx = (np.arange(128)[:, None] // 64) * np.ones(
            (1, KP), np.int64)
        ratom = amap[bidx, np.arange(KP)[None, :], jidx]
        ratom = np.where(val, ratom, 0)
        satom = np.where(val, sl, 0)
        pp = np.zeros((128, KP, 6), np.float32)
        pp[:, :, 0:3] = positions[satom][:, :, PERM]
        pp[:, :, 3:6] = positions[ratom][:, :, PERM]
        ce = np.zeros((32, KPC), np.float32)
        av = amap.reshape(2, KPC)
        for b in range(2):
            valid = av[b] >= 0
            ce[b * 16:(b + 1) * 16, valid] = cemb[av[b][valid]].T
        maps1.append(dict(pp=pp, wrb=WRB, gx=gx_pack(cemb, tb),
                          sw=SW1, ce=ce, ptz=PTZ))

    resA = bass_utils.run_bass_kernel_spmd(ncA, maps1,
                                           core_ids=list(range(NCORES)),
                                           trace=PROFILE)
    if PROFILE:
        LAST_PROF.append(resA)

    h1_full = np.zeros((N_ATOMS, C), np.float32)
    for core in range(NCORES):
        amap = tabs[core]["amap"].reshape(2, KPC)
        outh = resA.results[core]["outh"]  # [32, KPC]
        for b in range(2):
            valid = amap[b] >= 0
            h1_full[amap[b][valid]] = outh[b * 16:(b + 1) * 16, valid].T

    maps2 = []
    for core in range(NCORES):
        tb = tabs[core]
        maps2.append(dict(ptf=resA.results[core]["ptf"],
                          gx=gx_pack(h1_full, tb), se=SE2))
    resB = bass_utils.run_bass_kernel_spmd(ncB, maps2,
                                           core_ids=list(range(NCORES)),
                                           trace=PROFILE)
    if PROFILE:
        LAST_PROF.append(resB)

    e_atom = np.zeros(N_ATOMS, np.float32)
    for core in range(NCORES):
        amap = tabs[core]["amap"].reshape(2, KPC)
        oute = resB.results[core]["oute"]  # [2, KPC]
        for b in range(2):
            valid = amap[b] >= 0
            e_atom[amap[b][valid]] = oute[b, valid]
    e_atom += comp_weights[species]
    out = np.zeros(N_STRUCT, np.float32)
    np.add.at(out, structure_ids_np, e_atom)
    return out
